# revision 22
# baseline (speedup 1.0000x reference)
"""Fused graph Fokker-Planck ODE function kernel for Trainium2 (8 NeuronCores).

Sharding: data-parallel over batch B=4 x row-halves (i in [0,256) / [256,512))
-> 8 shards.  Each core computes dh_dt for one (batch, i-half) pair.

Math (per batch; [i,j] matrices kept transposed as [j,i] on chip so the
j-contraction matmuls need no transposes):
    S      = A * (K' @ Q^T)            K' = (pe Wk + bk)/sqrt(D) (host-folded)
    X      = exp(S)                    (unnormalized softmax)
    rd'    = -sigmoid(10*(E_i - E_j)) = -0.5*tanh(5*(E_i - E_j)) - 0.5
    M4n    = X * rd'                   (negated so X and M4n share one rhs)
Single-PSUM-chain accumulation over a shared packed rhs [h|EhL|1|EL'|1]:
    acc    = X^T @ rhs[:, 0:65]  +  M4n^T @ rhs[:, 0:98]
           = [G3h | A | r3 | -G4EL' | -r4]
    (EhL = E*h + beta*L*h, EL' = E + beta*L, L = log(h+1e-8) from host)
Finals:
    s = r3 + r4;  P = G3h + r4*h_i
    dh = (A - EL'_i * P + h_i * G4EL') / s
bf16/fp8 on the N^2 device path (fp32 PSUM accum); the O(N*PE*D) Q/K
projections and O(N*D) rhs packing run on host.  Measured rel err ~8e-4.
"""

import math
import sys

import numpy as np

for _p in ("/opt/trn_rl_repo",):
    if _p not in sys.path:
        sys.path.insert(0, _p)

import ml_dtypes

B, N, D, PED = 4, 512, 32, 16
NCORES = 8
RPC = N // 2            # i-rows per core
NJT = N // 128          # j tiles of 128
NIT = RPC // 128        # i tiles of 128
XW = 65                 # X-matmul rhs cols  [h|EhL|1]
MW = 98                 # M4n-matmul rhs cols [h|EhL|1|EL'|1]
TW = MW                 # packed rhs cols per j-tile (fully shared)
ISD = 1.0 / math.sqrt(D)
BF16 = ml_dtypes.bfloat16
FP8 = ml_dtypes.float8_e4m3

_CACHE = {}


def _patch_act_tables():
    """Make exp_and_others (exp + tanh + identity) the only ACT table set
    containing our functions so bacc emits exactly one ACT_TABLE_LOAD."""
    import concourse.bacc as bacc_mod
    if getattr(bacc_mod, "_act_tables_patched", False):
        return
    orig = bacc_mod.get_activation_tables

    def filtered(arch):
        t = orig(arch)
        target = t.get("exp_and_others")
        if not target:
            return t
        return {k: (v if k == "exp_and_others" else (v - target))
                for k, v in t.items()}

    bacc_mod.get_activation_tables = filtered
    bacc_mod._act_tables_patched = True


def _build_program():
    import concourse.bacc as bacc
    import concourse.tile as tile
    from concourse import mybir
    from contextlib import ExitStack

    _patch_act_tables()

    fp32 = mybir.dt.float32
    f32r = mybir.dt.float32r
    bf16 = mybir.dt.bfloat16
    fp8 = mybir.dt.float8e4
    AF = mybir.ActivationFunctionType
    ADD, MUL, SUB = (mybir.AluOpType.add, mybir.AluOpType.mult,
                     mybir.AluOpType.subtract)

    nc = bacc.Bacc("TRN2", target_bir_lowering=False, debug=False,
                   num_devices=NCORES)

    def din(name, shape, dt=fp32):
        return nc.dram_tensor(name, shape, dt, kind="ExternalInput").ap()

    smalls = din("smalls", [128, 8])            # [-5Ej(4) | pad]
    erow = din("erow", [1, RPC], f32r)          # E_i row (f32r for rank-1 MM)
    qkT = din("qkT", [D, N + RPC], bf16)        # [Q^T | K'^T], host-projected
    rhA = din("rhA", [128, 2 * TW], bf16)       # packed rhs j-tiles 0-1
    at01 = din("at01", [128, 2 * RPC], fp8)     # A[isl].T j-tiles 0-1
    at23 = din("at23", [128, 2 * RPC], fp8)     # A[isl].T j-tiles 2-3
    rhB = din("rhB", [128, 2 * TW], bf16)       # packed rhs j-tiles 2-3
    hili = din("hili", [128, 4 * D])            # [hi (NIT D) | EL'_i]
    out = nc.dram_tensor("out", [128, NIT * D + 2 * NIT], fp32,
                         kind="ExternalOutput").ap()

    with tile.TileContext(nc) as tc, ExitStack() as ctx:
        cst = ctx.enter_context(tc.tile_pool(name="cst", bufs=1))
        fin = ctx.enter_context(tc.tile_pool(name="fin", bufs=1))
        pp1 = ctx.enter_context(tc.tile_pool(name="pp1", bufs=1, space="PSUM"))

        # ---------------- input DMAs; sync takes the early-needed ----------
        erow_sb = cst.tile([1, RPC], f32r, tag="erow")
        nc.sync.dma_start(erow_sb[:], erow[:])
        smalls_sb = cst.tile([128, 8], fp32, tag="smalls")
        nc.sync.dma_start(smalls_sb[:], smalls[:])
        at_sb = cst.tile([128, NJT * RPC], fp8, tag="at_sb")
        nc.sync.dma_start(at_sb[:, 0:2 * RPC], at01[:])
        qkT_sb = cst.tile([D, N + RPC], bf16, tag="qkT")
        nc.sync.dma_start(qkT_sb[:], qkT[:])

        # V: constants (warm-act + eib deps)
        zero1 = cst.tile([128, 1], fp32, tag="zero1")
        nc.vector.memset(zero1[:], 0.0)
        ones1 = cst.tile([1, 128], f32r, tag="ones1")
        nc.vector.memset(ones1.bitcast(fp32)[:], 1.0)

        # rhs tiles 0-1 ride the fast sync queue after the critical loads;
        # big transfers issued early on other queues would contend for the
        # shared DMA engines and delay qkT/at01.
        rhsp_sb = cst.tile([128, NJT * TW], bf16, tag="rhsp")
        nc.sync.dma_start(rhsp_sb[:, 0:2 * TW], rhA[:])

        # scalar queue: ACT table + warm first (the table fetch hogs the
        # scalar DMA engine), then the late mask half and rhs tiles 2-3
        warm = cst.tile([128, 1], fp32, tag="warm")
        nc.scalar.activation(warm[:], zero1[:], AF.Exp, bias=zero1[:])
        nc.scalar.dma_start(at_sb[:, 2 * RPC:4 * RPC], at23[:])
        nc.scalar.dma_start(rhsp_sb[:, 2 * TW:4 * TW], rhB[:])

        # gpsimd queue (slow DMA path): finals-only tensor
        hili_sb = cst.tile([128, 4 * D], fp32, tag="hili")
        nc.gpsimd.dma_start(hili_sb[:], hili[:])

        m5ej = smalls_sb[:, 0:NJT]            # -5*E_j tiles
        qT = qkT_sb[:, 0:N]
        kT = qkT_sb[:, N:N + RPC]

        # accps allocated first => PSUM banks 0-1 (one aligned pair);
        # its single start=True clear covers both accumulation chains.
        accps = pp1.tile([128, 512], fp32, tag="accps")

        # ---------------- E_i broadcast (rank-1 f32r matmul) ---------------
        ek = pp1.tile([128, RPC], fp32, tag="ek")
        nc.tensor.matmul(ek[:], ones1[:], erow_sb[:], start=True, stop=True)
        eibps = ek[:]

        # ---------------- scores into one 4-bank PSUM tile -----------------
        sall = pp1.tile([128, NJT * RPC], fp32, tag="sall")
        for t in range(NJT):
            nc.tensor.matmul(sall[:, t * RPC:(t + 1) * RPC],
                             qT[:, t * 128:(t + 1) * 128], kT,
                             start=True, stop=True)

        tanh_sb = cst.tile([128, NJT * RPC], fp32, tag="tanh")
        rd_sb = cst.tile([128, NJT * RPC], bf16, tag="rd")
        msk_sb = cst.tile([128, NJT * RPC], fp32, tag="msk")
        X_sb = cst.tile([128, NJT * RPC], bf16, tag="X")
        M4_sb = cst.tile([128, NJT * RPC], bf16, tag="M4")

        def sl(t):
            return slice(t * RPC, (t + 1) * RPC)

        def dl(p):
            return slice(p * 2 * RPC, (p + 1) * 2 * RPC)

        # S queue: T0 T1 T2 T3 X01 X23; Pool rd' halves follow their tanhs
        for t in range(2):
            nc.scalar.activation(tanh_sb[:, sl(t)], eibps, AF.Tanh,
                                 bias=m5ej[:, t:t + 1], scale=5.0)
        nc.gpsimd.tensor_scalar(rd_sb[:, dl(0)], tanh_sb[:, dl(0)],
                                -0.5, -0.5, op0=MUL, op1=ADD)
        for t in range(2, NJT):
            nc.scalar.activation(tanh_sb[:, sl(t)], eibps, AF.Tanh,
                                 bias=m5ej[:, t:t + 1], scale=5.0)
        nc.gpsimd.tensor_scalar(rd_sb[:, dl(1)], tanh_sb[:, dl(1)],
                                -0.5, -0.5, op0=MUL, op1=ADD)
        nc.vector.tensor_tensor(msk_sb[:, dl(0)], at_sb[:, dl(0)],
                                sall[:, dl(0)], op=MUL)
        nc.scalar.activation(X_sb[:, dl(0)], msk_sb[:, dl(0)], AF.Exp,
                             bias=zero1[:])
        nc.vector.tensor_tensor(msk_sb[:, dl(1)], at_sb[:, dl(1)],
                                sall[:, dl(1)], op=MUL)
        nc.scalar.activation(X_sb[:, dl(1)], msk_sb[:, dl(1)], AF.Exp,
                             bias=zero1[:])
        nc.vector.tensor_tensor(M4_sb[:, dl(0)], X_sb[:, dl(0)],
                                rd_sb[:, dl(0)], op=MUL)
        nc.vector.tensor_tensor(M4_sb[:, dl(1)], X_sb[:, dl(1)],
                                rd_sb[:, dl(1)], op=MUL)

        # ---------------- shared-rhs accumulation matmuls ------------------
        # accps[:, it*256 + c]: c in [0:32) G3h, [32:64) A, 64 r3,
        # [65:97) -G4EL', 97 -r4.  A start=True clears has_written for the
        # whole bank pair, so exactly ONE start heads all four chains.
        first = True
        for t in range(NJT):
            sp = (t == NJT - 1)
            rs = rhsp_sb[:, t * TW:t * TW + TW]
            for it in range(NIT):
                xsl = X_sb[:, t * RPC + it * 128:t * RPC + (it + 1) * 128]
                msl = M4_sb[:, t * RPC + it * 128:t * RPC + (it + 1) * 128]
                nc.tensor.matmul(accps[:, it * 256:it * 256 + XW], xsl,
                                 rs[:, 0:XW], start=first, stop=sp)
                first = False
                nc.tensor.matmul(accps[:, it * 256:it * 256 + MW], msl, rs,
                                 start=False, stop=sp)

        # ---------------- finals (batched over both i-tiles) ---------------
        accv = accps.rearrange("p (t c) -> p t c", c=256)
        hiv = hili_sb[:, 0:NIT * D].rearrange("p (t d) -> p t d", d=D)
        eliv = hili_sb[:, NIT * D:2 * NIT * D].rearrange(
            "p (t d) -> p t d", d=D)

        vv = fin.tile([128, NIT, D], fp32, tag="vv")
        pp = fin.tile([128, NIT, D], fp32, tag="pp")
        t2 = fin.tile([128, NIT, D], fp32, tag="t2")
        qq = fin.tile([128, NIT, D], fp32, tag="qq")
        uu = fin.tile([128, NIT, D], fp32, tag="uu")
        res = fin.tile([128, NIT * D + 2 * NIT], fp32, tag="res")
        ddv = res[:, 0:NIT * D].rearrange("p (t d) -> p t d", d=D)

        # dd = A - EL'_i*P + hi*G4EL' (unnormalized); host divides by
        # s = r3 + r4 shipped in the last four output columns.
        nc.vector.tensor_tensor(vv[:], hiv[:],
                                accv[:, :, 97:98].to_broadcast((128, NIT, D)),
                                op=MUL)
        nc.vector.tensor_tensor(qq[:], hiv[:], accv[:, :, 65:65 + D], op=MUL)
        nc.vector.tensor_tensor(pp[:], accv[:, :, 0:D], vv[:], op=SUB)
        nc.vector.tensor_tensor(t2[:], eliv[:], pp[:], op=MUL)
        nc.vector.tensor_tensor(uu[:], accv[:, :, D:2 * D], t2[:], op=SUB)
        nc.vector.tensor_tensor(ddv[:], uu[:], qq[:], op=SUB)
        nc.vector.tensor_copy(
            res[:, NIT * D:NIT * D + NIT].rearrange("p (t o) -> p t o", o=1),
            accv[:, :, 64:65])
        nc.vector.tensor_copy(
            res[:, NIT * D + NIT:].rearrange("p (t o) -> p t o", o=1),
            accv[:, :, 97:98])
        nc.sync.dma_start(out[:], res[:])

    nc.compile()
    return nc


def _get_program():
    if "nc" not in _CACHE:
        _CACHE["nc"] = _build_program()
    return _CACHE["nc"]


def make_in_maps(h, pe, E, A, Wk, bk, Wq, bq, beta):
    f = lambda x: np.ascontiguousarray(np.asarray(x, dtype=np.float32))
    h, pe, E, A = f(h), f(pe), f(E), f(A)
    Wk, bk, Wq, bq, beta = f(Wk), f(bk), f(Wq), f(bq), f(beta)

    L = np.log(h + 1e-8)                                    # [B,N,D]
    lip = beta[None, None, :] * L                           # beta*L
    in_maps = []
    ones_col = np.ones((N, 1), np.float32)
    rhs_c, q_c = {}, {}
    for b in range(B):
        EhL = E[:, None] * h[b] + lip[b] * h[b]
        ELp = E[:, None] + lip[b]
        # [h | EhL | 1 | EL' | 1]  -> [N, 98]
        R = np.concatenate([h[b], EhL, ones_col, ELp, ones_col], axis=1)
        rhs_c[b] = np.ascontiguousarray(
            R.reshape(NJT, 128, TW).transpose(1, 0, 2).reshape(128, NJT * TW)
        ).astype(BF16)
        q_c[b] = np.ascontiguousarray((pe[b] @ Wq + bq).T)      # [D, N]
    for c in range(NCORES):
        b, r = c // 2, c % 2
        isl = slice(r * RPC, (r + 1) * RPC)
        atp = A[isl].T.reshape(NJT, 128, RPC).transpose(1, 0, 2).reshape(
            128, NJT * RPC).astype(FP8)
        kT = ((pe[b, isl] @ Wk + bk) * ISD).T                   # [D, RPC]
        qkT = np.concatenate([q_c[b], kT], axis=1).astype(BF16)
        smalls = np.zeros((128, 8), np.float32)
        smalls[:, 0:NJT] = -5.0 * E.reshape(NJT, 128).T
        hili = np.empty((128, 4 * D), np.float32)
        hili[:, 0:NIT * D] = h[b, isl].reshape(NIT, 128, D).transpose(
            1, 0, 2).reshape(128, NIT * D)
        ELp_b = E[isl, None] + lip[b, isl]
        hili[:, NIT * D:] = ELp_b.reshape(NIT, 128, D).transpose(
            1, 0, 2).reshape(128, NIT * D)
        in_maps.append({
            "smalls": smalls,
            "erow": E[isl].reshape(1, RPC).copy(),
            "qkT": qkT,
            "rhA": np.ascontiguousarray(rhs_c[b][:, 0:2 * TW]),
            "at01": np.ascontiguousarray(atp[:, 0:2 * RPC]),
            "at23": np.ascontiguousarray(atp[:, 2 * RPC:4 * RPC]),
            "rhB": np.ascontiguousarray(rhs_c[b][:, 2 * TW:4 * TW]),
            "hili": hili,
        })
    return in_maps


def gather(results):
    out = np.empty((B, N, D), np.float32)
    for c in range(NCORES):
        b, r = c // 2, c % 2
        raw = results[c]["out"]
        dd = raw[:, 0:NIT * D].reshape(128, NIT, D)
        r3 = raw[:, NIT * D:NIT * D + NIT]
        r4n = raw[:, NIT * D + NIT:]
        s = r3 - r4n                                  # [128, NIT]
        o = (dd / s[:, :, None]).transpose(1, 0, 2)
        out[b, r * RPC:(r + 1) * RPC] = o.reshape(RPC, D)
    return out


def _axon_reset():
    try:
        import ctypes
        import jax
        lib = ctypes.CDLL("/opt/axon/libaxon_pjrt.so")
        lib.axon_reset.restype = ctypes.c_int64
        jax.devices()
        lib.axon_reset()
    except Exception:
        pass


def kernel(t=None, h=None, pe=None, E=None, A=None, Wk=None, bk=None,
           Wq=None, bq=None, beta=None, **_unused):
    from concourse.bass_utils import run_bass_kernel_spmd
    nc = _get_program()
    in_maps = make_in_maps(h, pe, E, A, Wk, bk, Wq, bq, beta)
    try:
        res = run_bass_kernel_spmd(nc, in_maps, list(range(NCORES)))
    except Exception:
        # a previously wedged NeuronCore shows up as an opaque runtime
        # error on the first execute — reset the device once and retry
        _axon_reset()
        import time as _time
        _time.sleep(2)
        res = run_bass_kernel_spmd(nc, in_maps, list(range(NCORES)))
    return gather(res.results)


# revision 23
# speedup vs baseline: 1.0397x; 1.0397x over previous
"""Fused graph Fokker-Planck ODE function kernel for Trainium2 (8 NeuronCores).

Sharding: data-parallel over batch B=4 x row-halves (i in [0,256) / [256,512))
-> 8 shards.  Each core computes dh_dt for one (batch, i-half) pair.

Math (per batch; [i,j] matrices kept transposed as [j,i] on chip so the
j-contraction matmuls need no transposes):
    S      = A * (K' @ Q^T)            K' = (pe Wk + bk)/sqrt(D) (host-folded)
    X      = exp(S)                    (unnormalized softmax)
    rd'    = -sigmoid(10*(E_i - E_j)) = -0.5*tanh(5*(E_i - E_j)) - 0.5
    M4n    = X * rd'                   (negated so X and M4n share one rhs)
Single-PSUM-chain accumulation over a shared packed rhs [h|EhL|1|EL'|1]:
    acc    = X^T @ rhs[:, 0:65]  +  M4n^T @ rhs[:, 0:98]
           = [G3h | A | r3 | -G4EL' | -r4]
    (EhL = E*h + beta*L*h, EL' = E + beta*L, L = log(h+1e-8) from host)
Finals:
    s = r3 + r4;  P = G3h + r4*h_i
    dh = (A - EL'_i * P + h_i * G4EL') / s
bf16/fp8 on the N^2 device path (fp32 PSUM accum); the O(N*PE*D) Q/K
projections and O(N*D) rhs packing run on host.  Measured rel err ~8e-4.
"""

import math
import sys

import numpy as np

for _p in ("/opt/trn_rl_repo",):
    if _p not in sys.path:
        sys.path.insert(0, _p)

import ml_dtypes

B, N, D, PED = 4, 512, 32, 16
NCORES = 8
RPC = N // 2            # i-rows per core
NJT = N // 128          # j tiles of 128
NIT = RPC // 128        # i tiles of 128
XW = 65                 # X-matmul rhs cols  [h|EhL|1]
MW = 98                 # M4n-matmul rhs cols [h|EhL|1|EL'|1]
TW = MW                 # packed rhs cols per j-tile (fully shared)
ISD = 1.0 / math.sqrt(D)
BF16 = ml_dtypes.bfloat16
FP8 = ml_dtypes.float8_e4m3

_CACHE = {}


def _patch_act_tables():
    """Make exp_and_others (exp + tanh + identity) the only ACT table set
    containing our functions so bacc emits exactly one ACT_TABLE_LOAD."""
    import concourse.bacc as bacc_mod
    if getattr(bacc_mod, "_act_tables_patched", False):
        return
    orig = bacc_mod.get_activation_tables

    def filtered(arch):
        t = orig(arch)
        target = t.get("exp_and_others")
        if not target:
            return t
        return {k: (v if k == "exp_and_others" else (v - target))
                for k, v in t.items()}

    bacc_mod.get_activation_tables = filtered
    bacc_mod._act_tables_patched = True


def _build_program():
    import concourse.bacc as bacc
    import concourse.tile as tile
    from concourse import mybir
    from contextlib import ExitStack

    _patch_act_tables()

    fp32 = mybir.dt.float32
    f32r = mybir.dt.float32r
    bf16 = mybir.dt.bfloat16
    fp8 = mybir.dt.float8e4
    AF = mybir.ActivationFunctionType
    ADD, MUL, SUB = (mybir.AluOpType.add, mybir.AluOpType.mult,
                     mybir.AluOpType.subtract)

    nc = bacc.Bacc("TRN2", target_bir_lowering=False, debug=False,
                   num_devices=NCORES)

    def din(name, shape, dt=fp32):
        return nc.dram_tensor(name, shape, dt, kind="ExternalInput").ap()

    smalls = din("smalls", [128, 8])            # [-5Ej(4) | pad]
    erow = din("erow", [1, RPC], f32r)          # E_i row (f32r for rank-1 MM)
    qkT = din("qkT", [D, N + RPC], bf16)        # [Q^T | K'^T], host-projected
    rhA = din("rhA", [128, 2 * TW], bf16)       # packed rhs j-tiles 0-1
    at01 = din("at01", [128, 2 * RPC], fp8)     # A[isl].T j-tiles 0-1
    at23 = din("at23", [128, 2 * RPC], fp8)     # A[isl].T j-tiles 2-3
    rhB = din("rhB", [128, 2 * TW], bf16)       # packed rhs j-tiles 2-3
    hili = din("hili", [128, 6 * D])            # [hi | EL'_i | EL'_i*h_i]
    out = nc.dram_tensor("out", [128, NIT * D + 2 * NIT], fp32,
                         kind="ExternalOutput").ap()

    with tile.TileContext(nc) as tc, ExitStack() as ctx:
        cst = ctx.enter_context(tc.tile_pool(name="cst", bufs=1))
        fin = ctx.enter_context(tc.tile_pool(name="fin", bufs=1))
        pp1 = ctx.enter_context(tc.tile_pool(name="pp1", bufs=1, space="PSUM"))

        # ---------------- input DMAs; sync takes the early-needed ----------
        erow_sb = cst.tile([1, RPC], f32r, tag="erow")
        nc.sync.dma_start(erow_sb[:], erow[:])
        smalls_sb = cst.tile([128, 8], fp32, tag="smalls")
        nc.sync.dma_start(smalls_sb[:], smalls[:])
        qkT_sb = cst.tile([D, N + RPC], bf16, tag="qkT")
        nc.sync.dma_start(qkT_sb[:], qkT[:])
        at_sb = cst.tile([128, NJT * RPC], fp8, tag="at_sb")
        nc.sync.dma_start(at_sb[:, 0:2 * RPC], at01[:])

        # V: constants (warm-act + eib deps)
        zero1 = cst.tile([128, 1], fp32, tag="zero1")
        nc.vector.memset(zero1[:], 0.0)
        ones1 = cst.tile([1, 128], f32r, tag="ones1")
        nc.vector.memset(ones1.bitcast(fp32)[:], 1.0)

        # rhs tiles 0-1 ride the fast sync queue after the critical loads;
        # big transfers issued early on other queues would contend for the
        # shared DMA engines and delay qkT/at01.
        rhsp_sb = cst.tile([128, NJT * TW], bf16, tag="rhsp")
        nc.sync.dma_start(rhsp_sb[:, 0:2 * TW], rhA[:])

        # scalar queue: ACT table + warm first (the table fetch hogs the
        # scalar DMA engine), then the late mask half and rhs tiles 2-3
        warm = cst.tile([128, 1], fp32, tag="warm")
        nc.scalar.activation(warm[:], zero1[:], AF.Exp, bias=zero1[:])
        nc.scalar.dma_start(at_sb[:, 2 * RPC:4 * RPC], at23[:])
        nc.scalar.dma_start(rhsp_sb[:, 2 * TW:4 * TW], rhB[:])

        # gpsimd queue (slow DMA path): finals-only tensor
        hili_sb = cst.tile([128, 6 * D], fp32, tag="hili")
        nc.gpsimd.dma_start(hili_sb[:], hili[:])

        m5ej = smalls_sb[:, 0:NJT]            # -5*E_j tiles
        qT = qkT_sb[:, 0:N]
        kT = qkT_sb[:, N:N + RPC]

        # accps allocated first => PSUM banks 0-1 (one aligned pair);
        # its single start=True clear covers both accumulation chains.
        accps = pp1.tile([128, 512], fp32, tag="accps")

        # ---------------- E_i broadcast (rank-1 f32r matmul) ---------------
        ek = pp1.tile([128, RPC], fp32, tag="ek")
        nc.tensor.matmul(ek[:], ones1[:], erow_sb[:], start=True, stop=True)
        eibps = ek[:]

        # ---------------- scores into one 4-bank PSUM tile -----------------
        sall = pp1.tile([128, NJT * RPC], fp32, tag="sall")
        for t in range(NJT):
            nc.tensor.matmul(sall[:, t * RPC:(t + 1) * RPC],
                             qT[:, t * 128:(t + 1) * 128], kT,
                             start=True, stop=True)

        tanh_sb = cst.tile([128, NJT * RPC], fp32, tag="tanh")
        rd_sb = cst.tile([128, NJT * RPC], bf16, tag="rd")
        msk_sb = cst.tile([128, NJT * RPC], fp32, tag="msk")
        X_sb = cst.tile([128, NJT * RPC], bf16, tag="X")
        M4_sb = cst.tile([128, NJT * RPC], bf16, tag="M4")

        def sl(t):
            return slice(t * RPC, (t + 1) * RPC)

        def dl(p):
            return slice(p * 2 * RPC, (p + 1) * 2 * RPC)

        # S queue: T0 T1 T2 T3 X01 X23; Pool rd' halves follow their tanhs
        for t in range(2):
            nc.scalar.activation(tanh_sb[:, sl(t)], eibps, AF.Tanh,
                                 bias=m5ej[:, t:t + 1], scale=5.0)
        nc.gpsimd.tensor_scalar(rd_sb[:, dl(0)], tanh_sb[:, dl(0)],
                                -0.5, -0.5, op0=MUL, op1=ADD)
        for t in range(2, NJT):
            nc.scalar.activation(tanh_sb[:, sl(t)], eibps, AF.Tanh,
                                 bias=m5ej[:, t:t + 1], scale=5.0)
        nc.gpsimd.tensor_scalar(rd_sb[:, dl(1)], tanh_sb[:, dl(1)],
                                -0.5, -0.5, op0=MUL, op1=ADD)
        nc.vector.tensor_tensor(msk_sb[:, dl(0)], at_sb[:, dl(0)],
                                sall[:, dl(0)], op=MUL)
        nc.scalar.activation(X_sb[:, dl(0)], msk_sb[:, dl(0)], AF.Exp,
                             bias=zero1[:])
        nc.vector.tensor_tensor(msk_sb[:, dl(1)], at_sb[:, dl(1)],
                                sall[:, dl(1)], op=MUL)
        nc.scalar.activation(X_sb[:, dl(1)], msk_sb[:, dl(1)], AF.Exp,
                             bias=zero1[:])
        nc.vector.tensor_tensor(M4_sb[:, dl(0)], X_sb[:, dl(0)],
                                rd_sb[:, dl(0)], op=MUL)
        nc.vector.tensor_tensor(M4_sb[:, dl(1)], X_sb[:, dl(1)],
                                rd_sb[:, dl(1)], op=MUL)

        # ---------------- shared-rhs accumulation matmuls ------------------
        # accps[:, it*256 + c]: c in [0:32) G3h, [32:64) A, 64 r3,
        # [65:97) -G4EL', 97 -r4.  A start=True clears has_written for the
        # whole bank pair, so exactly ONE start heads all four chains.
        first = True
        for t in range(NJT):
            sp = (t == NJT - 1)
            rs = rhsp_sb[:, t * TW:t * TW + TW]
            for it in range(NIT):
                xsl = X_sb[:, t * RPC + it * 128:t * RPC + (it + 1) * 128]
                msl = M4_sb[:, t * RPC + it * 128:t * RPC + (it + 1) * 128]
                nc.tensor.matmul(accps[:, it * 256:it * 256 + XW], xsl,
                                 rs[:, 0:XW], start=first, stop=sp)
                first = False
                nc.tensor.matmul(accps[:, it * 256:it * 256 + MW], msl, rs,
                                 start=False, stop=sp)

        # ---------------- finals (batched over both i-tiles) ---------------
        accv = accps.rearrange("p (t c) -> p t c", c=256)
        hiv = hili_sb[:, 0:NIT * D].rearrange("p (t d) -> p t d", d=D)
        eliv = hili_sb[:, NIT * D:2 * NIT * D].rearrange(
            "p (t d) -> p t d", d=D)
        ehiv = hili_sb[:, 2 * NIT * D:3 * NIT * D].rearrange(
            "p (t d) -> p t d", d=D)

        a1 = fin.tile([128, NIT, D], fp32, tag="a1")
        a2 = fin.tile([128, NIT, D], fp32, tag="a2")
        qq = fin.tile([128, NIT, D], fp32, tag="qq")
        c1 = fin.tile([128, NIT, D], fp32, tag="c1")
        c2 = fin.tile([128, NIT, D], fp32, tag="c2")
        res = fin.tile([128, NIT * D + 2 * NIT], fp32, tag="res")
        ddv = res[:, 0:NIT * D].rearrange("p (t d) -> p t d", d=D)

        # dd = A - EL'i*G3h + (EL'i*hi)*(-r4) - hi*(-G4EL') (unnormalized,
        # independent products flatten the dependency chain); host divides
        # by s = r3 + r4 shipped in the last four output columns.
        nc.vector.tensor_tensor(a1[:], eliv[:], accv[:, :, 0:D], op=MUL)
        nc.vector.tensor_tensor(a2[:], ehiv[:],
                                accv[:, :, 97:98].to_broadcast((128, NIT, D)),
                                op=MUL)
        nc.vector.tensor_tensor(qq[:], hiv[:], accv[:, :, 65:65 + D], op=MUL)
        nc.vector.tensor_copy(
            res[:, NIT * D:NIT * D + NIT].rearrange("p (t o) -> p t o", o=1),
            accv[:, :, 64:65])
        nc.vector.tensor_copy(
            res[:, NIT * D + NIT:].rearrange("p (t o) -> p t o", o=1),
            accv[:, :, 97:98])
        nc.vector.tensor_tensor(c1[:], accv[:, :, D:2 * D], a1[:], op=SUB)
        nc.vector.tensor_tensor(c2[:], a2[:], qq[:], op=SUB)
        nc.vector.tensor_tensor(ddv[:], c1[:], c2[:], op=ADD)
        nc.sync.dma_start(out[:], res[:])

    nc.compile()
    return nc


def _get_program():
    if "nc" not in _CACHE:
        _CACHE["nc"] = _build_program()
    return _CACHE["nc"]


def make_in_maps(h, pe, E, A, Wk, bk, Wq, bq, beta):
    f = lambda x: np.ascontiguousarray(np.asarray(x, dtype=np.float32))
    h, pe, E, A = f(h), f(pe), f(E), f(A)
    Wk, bk, Wq, bq, beta = f(Wk), f(bk), f(Wq), f(bq), f(beta)

    L = np.log(h + 1e-8)                                    # [B,N,D]
    lip = beta[None, None, :] * L                           # beta*L
    in_maps = []
    ones_col = np.ones((N, 1), np.float32)
    rhs_c, q_c = {}, {}
    for b in range(B):
        EhL = E[:, None] * h[b] + lip[b] * h[b]
        ELp = E[:, None] + lip[b]
        # [h | EhL | 1 | EL' | 1]  -> [N, 98]
        R = np.concatenate([h[b], EhL, ones_col, ELp, ones_col], axis=1)
        rhs_c[b] = np.ascontiguousarray(
            R.reshape(NJT, 128, TW).transpose(1, 0, 2).reshape(128, NJT * TW)
        ).astype(BF16)
        q_c[b] = np.ascontiguousarray((pe[b] @ Wq + bq).T)      # [D, N]
    for c in range(NCORES):
        b, r = c // 2, c % 2
        isl = slice(r * RPC, (r + 1) * RPC)
        atp = A[isl].T.reshape(NJT, 128, RPC).transpose(1, 0, 2).reshape(
            128, NJT * RPC).astype(FP8)
        kT = ((pe[b, isl] @ Wk + bk) * ISD).T                   # [D, RPC]
        qkT = np.concatenate([q_c[b], kT], axis=1).astype(BF16)
        smalls = np.zeros((128, 8), np.float32)
        smalls[:, 0:NJT] = -5.0 * E.reshape(NJT, 128).T
        hili = np.empty((128, 6 * D), np.float32)
        hili[:, 0:NIT * D] = h[b, isl].reshape(NIT, 128, D).transpose(
            1, 0, 2).reshape(128, NIT * D)
        ELp_b = E[isl, None] + lip[b, isl]
        hili[:, NIT * D:2 * NIT * D] = ELp_b.reshape(NIT, 128, D).transpose(
            1, 0, 2).reshape(128, NIT * D)
        hili[:, 2 * NIT * D:] = (ELp_b * h[b, isl]).reshape(
            NIT, 128, D).transpose(1, 0, 2).reshape(128, NIT * D)
        in_maps.append({
            "smalls": smalls,
            "erow": E[isl].reshape(1, RPC).copy(),
            "qkT": qkT,
            "rhA": np.ascontiguousarray(rhs_c[b][:, 0:2 * TW]),
            "at01": np.ascontiguousarray(atp[:, 0:2 * RPC]),
            "at23": np.ascontiguousarray(atp[:, 2 * RPC:4 * RPC]),
            "rhB": np.ascontiguousarray(rhs_c[b][:, 2 * TW:4 * TW]),
            "hili": hili,
        })
    return in_maps


def gather(results):
    out = np.empty((B, N, D), np.float32)
    for c in range(NCORES):
        b, r = c // 2, c % 2
        raw = results[c]["out"]
        dd = raw[:, 0:NIT * D].reshape(128, NIT, D)
        r3 = raw[:, NIT * D:NIT * D + NIT]
        r4n = raw[:, NIT * D + NIT:]
        s = r3 - r4n                                  # [128, NIT]
        o = (dd / s[:, :, None]).transpose(1, 0, 2)
        out[b, r * RPC:(r + 1) * RPC] = o.reshape(RPC, D)
    return out


def _axon_reset():
    try:
        import ctypes
        import jax
        lib = ctypes.CDLL("/opt/axon/libaxon_pjrt.so")
        lib.axon_reset.restype = ctypes.c_int64
        jax.devices()
        lib.axon_reset()
    except Exception:
        pass


def kernel(t=None, h=None, pe=None, E=None, A=None, Wk=None, bk=None,
           Wq=None, bq=None, beta=None, **_unused):
    from concourse.bass_utils import run_bass_kernel_spmd
    nc = _get_program()
    in_maps = make_in_maps(h, pe, E, A, Wk, bk, Wq, bq, beta)
    try:
        res = run_bass_kernel_spmd(nc, in_maps, list(range(NCORES)))
    except Exception:
        # a previously wedged NeuronCore shows up as an opaque runtime
        # error on the first execute — reset the device once and retry
        _axon_reset()
        import time as _time
        _time.sleep(2)
        res = run_bass_kernel_spmd(nc, in_maps, list(range(NCORES)))
    return gather(res.results)
